# revision 6
# baseline (speedup 1.0000x reference)
"""Liquid State Machine kernel for Trainium2, 8 NeuronCores — v4.

v2 (fp8-direct spike transport) + the latency tricks validated in sim:
- Semaphore bounces: cross-engine waits on DMA/collective completion sems
  observe ~1-1.7us late; a same-engine wait_ge on the completion sem chained
  with .then_inc(fast_sem) releases at completion and propagates in ~100ns.
  Applied to stage-out -> collective, collective -> gather DMAs, and
  gather DMAs -> PE.
- Threshold flip: spikes = (I >= 1 - alpha*u) with THM = 1 - alpha*u''
  precomputed off-critical, so the only DVE op between the last matmul and
  the stage-out DMA is one tensor_tensor is_ge (fp8 out). The membrane
  update (add/reset/decay) runs after the spikes are already in flight.
- Gather DMA split: slots 0-3 on the Activation HWDGE queue, slots 4-7 on
  the SP queue, each ~64KB, with per-half PE gating.
- Spikes cross cores as fp8_e4m3 {0,1} and feed the matmul moving operand
  directly against fp16 stationary weights (mixed-dtype matmul); weights are
  2-term fp16 splits (measured rate rel err 2.5e-3 on this instance).
"""

import numpy as np

from contextlib import ExitStack

import concourse.bass as bass
import concourse.mybir as mybir

ALPHA = 0.9
THRESHOLD = 1.0
B, T, D, R = 64, 256, 512, 2048
NCORES = 8
RC = R // NCORES          # 256 neurons per core
NT = 2                    # fp16 split terms for W_rec / W_in
WDT = mybir.dt.float16
NPW = np.float16
F8 = mybir.dt.float8e4
F32 = mybir.dt.float32
AOT = mybir.AluOpType
XBUF = 4


def build_lsm_nc(nsteps=T, nt=NT):
    nc = bass.Bass(num_devices=NCORES)

    wr = [nc.dram_tensor(f"wr{i}", [16, 128, RC], WDT, kind="ExternalInput")
          for i in range(nt)]
    wi = [nc.dram_tensor(f"wi{i}", [4, 128, RC], WDT, kind="ExternalInput")
          for i in range(nt)]
    xt = nc.dram_tensor("xt", [nsteps, 128, 256], WDT, kind="ExternalInput")
    out = nc.dram_tensor("out", [128, 128], F32, kind="ExternalOutput")
    bin_ = nc.dram_tensor("bounce_in", [128, 64], F8)
    bout = nc.dram_tensor("bounce_out", [NCORES * 128, 64], F8)

    nc.all_core_barrier()

    with ExitStack() as ctx:
        WR = ctx.enter_context(nc.sbuf_tensor("WR", [128, nt * 16 * RC], WDT))
        WI = ctx.enter_context(nc.sbuf_tensor("WI", [128, nt * 4 * RC], WDT))
        SPK8 = ctx.enter_context(nc.sbuf_tensor("SPK8", [128, 2 * 1024], F8))
        XT = ctx.enter_context(nc.sbuf_tensor("XT", [128, XBUF * 256], WDT))
        U = ctx.enter_context(nc.sbuf_tensor("U", [128, 128], F32))
        KEEP = ctx.enter_context(nc.sbuf_tensor("KEEP", [128, 128], F32))
        ACC = ctx.enter_context(nc.sbuf_tensor("ACC", [128, 128], F32))
        OUTS = ctx.enter_context(nc.sbuf_tensor("OUTS", [128, 128], F32))
        THM = ctx.enter_context(nc.sbuf_tensor("THM", [128, 128], F32))
        STGU = ctx.enter_context(nc.sbuf_tensor("STGU", [128, 2 * 128], F8))
        PACK8 = ctx.enter_context(nc.sbuf_tensor("PACK8", [128, 2 * 64], F8))
        SPKP = ctx.enter_context(nc.sbuf_tensor("SPKP", [128, 2 * 512], F8))
        PS00 = ctx.enter_context(nc.psum_tensor("PS00", [128, 64], F32))
        PS01 = ctx.enter_context(nc.psum_tensor("PS01", [128, 64], F32))
        PS10 = ctx.enter_context(nc.psum_tensor("PS10", [128, 64], F32))
        PS11 = ctx.enter_context(nc.psum_tensor("PS11", [128, 64], F32))
        sems = {}
        for s in ("sem_w sem_wa sem_fin sem_mm0 sem_mm1 sem_dve sem_u8 sem_stgd "
                  "cc_sem sem_init sem_x0 sem_x1 sem_x2 sem_x3 "
                  "sem_stg2 cc2 sem_gina sem_ginb sem_gxa sem_gxb sem_stgb sem_unp sem_unp2"
                  ).split():
            sems[s] = ctx.enter_context(nc.semaphore(s))
        sem_w, sem_wa, sem_fin = sems["sem_w"], sems["sem_wa"], sems["sem_fin"]
        sem_mm0, sem_mm1 = sems["sem_mm0"], sems["sem_mm1"]
        sem_dve, sem_u8, sem_stgd = sems["sem_dve"], sems["sem_u8"], sems["sem_stgd"]
        cc_sem, sem_init = sems["cc_sem"], sems["sem_init"]
        sem_xb = [sems[f"sem_x{i}"] for i in range(4)]
        sem_stg2, cc2 = sems["sem_stg2"], sems["cc2"]
        sem_gina, sem_ginb = sems["sem_gina"], sems["sem_ginb"]
        sem_gxa, sem_gxb = sems["sem_gxa"], sems["sem_gxb"]
        sem_stgb = sems["sem_stgb"]
        sem_unp = sems["sem_unp"]
        sem_unp2 = sems["sem_unp2"]
        PS = [[PS00, PS01], [PS10, PS11]]

        def wr_tile(term, q, mm):          # lhsT [128, 128] for W_rec block q
            base = (term * 16 + q) * RC + mm * 128
            return WR[:, base:base + 128]

        def wi_tile(term, dd, mm):
            base = (term * 4 + dd) * RC + mm * 128
            return WI[:, base:base + 128]

        def spk_rhs(buf, j, kk):           # [128, 64] fp8 moving operand
            base = buf * 1024 + j * 128 + kk * 64
            return SPK8[:, base:base + 64]

        def xt_rhs(tb, dd):
            return XT[:, tb * 256 + dd * 64: tb * 256 + dd * 64 + 64]

        with nc.Block() as block:

            @block.sync
            def _(sync):
                for i in range(nt):
                    for dd in range(4):
                        sync.dma_start(
                            WI[:, (i * 4 + dd) * RC:(i * 4 + dd + 1) * RC],
                            wi[i][dd, :, :],
                        ).then_inc(sem_wa, 16)
                for i in range(nt):
                    for q in range(16):
                        sync.dma_start(
                            WR[:, (i * 16 + q) * RC:(i * 16 + q + 1) * RC],
                            wr[i][q, :, :],
                        ).then_inc(sem_w, 16)
                for t in range(min(XBUF, nsteps)):
                    sync.dma_start(
                        XT[:, (t % XBUF) * 256:(t % XBUF) * 256 + 256],
                        xt[t, :, :],
                    ).then_inc(sem_xb[t % XBUF], 16)
                for t in range(nsteps):
                    if t < nsteps - 1:
                        # stage-out half 0 (half 1 rides the Act queue)
                        sync.wait_ge(sem_u8, 2 * t + 1)
                        sync.dma_start(
                            bin_[:, 0:32],
                            PACK8[:, (t % 2) * 64:(t % 2) * 64 + 32],
                        ).then_inc(sem_stgd, 16)
                        sync.wait_ge(sem_stgd, 16 * (t + 1)).then_inc(sem_stg2, 1)
                        # gather-in slots 4-7 (slots 0-3 ride the Act queue)
                        sync.wait_ge(cc2, 2 * (t + 1))
                        sync.dma_start(
                            SPKP[:, (t % 2) * 512 + 256:(t % 2) * 512 + 512],
                            bout.ap().rearrange(
                                "(j p) n -> p j n", p=128)[:, 4:8, :],
                        ).then_inc(sem_ginb, 16)
                        sync.wait_ge(sem_ginb, 16 * (t + 1)).then_inc(sem_gxb, 1)
                    tp = t + XBUF
                    if tp < nsteps:
                        sync.wait_ge(sem_mm1, tp - XBUF + 1)
                        sync.dma_start(
                            XT[:, (tp % XBUF) * 256:(tp % XBUF) * 256 + 256],
                            xt[tp, :, :],
                        ).then_inc(sem_xb[tp % XBUF], 16)
                sync.wait_ge(sem_init, 1)
                sync.dma_start(out[:, :], OUTS[:, :]).then_inc(sem_fin, 16)
                sync.wait_ge(sem_fin, 16)

            @block.gpsimd
            def _(g):
                for t in range(nsteps - 1):
                    g.wait_ge(sem_stg2, 2 * (t + 1))
                    g.collective_compute(
                        "AllGather",
                        mybir.AluOpType.bypass,
                        replica_groups=[list(range(NCORES))],
                        ins=[bin_.ap().opt()],
                        outs=[bout.ap().opt()],
                    ).then_inc(cc_sem, 1)
                    g.wait_ge(cc_sem, t + 1).then_inc(cc2, 2)

            @block.scalar
            def _(act):
                for t in range(nsteps - 1):
                    # stage-out half 1
                    act.wait_ge(sem_u8, 2 * t + 2)
                    act.dma_start(
                        bin_[:, 32:64],
                        PACK8[:, (t % 2) * 64 + 32:(t % 2) * 64 + 64],
                    ).then_inc(sem_stgb, 16)
                    act.wait_ge(sem_stgb, 16 * (t + 1)).then_inc(sem_stg2, 1)
                    # gather-in slots 0-3
                    act.wait_ge(cc2, 2 * (t + 1))
                    act.dma_start(
                        SPKP[:, (t % 2) * 512:(t % 2) * 512 + 256],
                        bout.ap().rearrange("(j p) n -> p j n", p=128)[:, 0:4, :],
                    ).then_inc(sem_gina, 16)
                    act.wait_ge(sem_gina, 16 * (t + 1)).then_inc(sem_gxa, 1)

            @block.tensor
            def _(tensor):

                def w_in(tau):
                    lw = [None, None]
                    tensor.wait_ge(sem_xb[tau % XBUF], 16 * (tau // XBUF + 1))
                    for mm in range(2):
                        if tau >= 2:
                            tensor.wait_ge(sem_dve, 2 * (tau - 2) + mm + 1)
                        for dd in range(4):
                            for i in range(nt):
                                lw[mm] = tensor.matmul(
                                    PS[tau % 2][mm][:, :],
                                    wi_tile(i, dd, mm),
                                    xt_rhs(tau % XBUF, dd),
                                    start=(dd == 0 and i == 0),
                                    stop=(tau == 0 and dd == 3 and i == nt - 1),
                                )
                    return lw

                tensor.wait_ge(sem_wa, 16 * nt * 4)
                last = w_in(0)
                for t in range(nsteps):
                    buf = t % 2
                    ps = PS[buf]
                    if t >= 1:
                        if t == 1:
                            tensor.wait_ge(sem_w, 16 * nt * 16)
                        for mm in range(2):
                            for j in range(NCORES):
                                if mm == 0 and j == 0:
                                    tensor.wait_ge(sem_unp, t)
                                if mm == 0 and j == 4:
                                    tensor.wait_ge(sem_unp2, t)
                                for kk in range(2):
                                    q = 2 * j + kk
                                    for i in range(nt):
                                        last[mm] = tensor.matmul(
                                            ps[mm][:, :],
                                            wr_tile(i, q, mm),
                                            spk_rhs((t - 1) % 2, j, kk),
                                            start=False,
                                            stop=(kk == 1 and j == NCORES - 1
                                                  and i == nt - 1),
                                        )
                    last[0].then_inc(sem_mm0, 1)
                    last[1].then_inc(sem_mm1, 1)
                    if t + 1 < nsteps:
                        tensor.wait_ge(cc2, 2 * (t + 1))
                        last = w_in(t + 1)

            @block.vector
            def _(vector):
                vector.memset(U[:, :], 0.0)
                vector.memset(ACC[:, :], 0.0)
                vector.memset(THM[:, :], THRESHOLD)
                vector.drain()
                for t in range(nsteps):
                    ps = PS[t % 2]
                    # critical: one is_ge per half (fp8 out), straight to DMA
                    if t >= 2:
                        vector.wait_ge(sem_stg2, 2 * (t - 1))
                    for h in range(2):
                        cols = slice(h * 64, h * 64 + 64)
                        sb = (t % 2) * 128 + h * 64
                        vector.wait_ge([sem_mm0, sem_mm1][h], t + 1)
                        vector.tensor_tensor(
                            STGU[:, sb:sb + 64],
                            ps[h][:, :], THM[:, cols], AOT.is_ge,
                        )
                        vector.drain()
                        # pack 2 spikes/byte: v = s_even + 2*s_odd in {0..3}
                        pb = (t % 2) * 64 + h * 32
                        vector.scalar_tensor_tensor(
                            PACK8[:, pb:pb + 32],
                            STGU[:, sb + 1:sb + 64:2], 2.0,
                            STGU[:, sb:sb + 64:2],
                            AOT.mult, AOT.add,
                        ).then_inc(sem_u8, 1)
                    # off-critical: membrane update + next threshold + count
                    for h in range(2):
                        cols = slice(h * 64, h * 64 + 64)
                        vector.tensor_add(
                            U[:, cols], U[:, cols], ps[h][:, :]
                        ).then_inc(sem_dve, 1)
                        vector.drain()
                        vector.tensor_scalar(
                            KEEP[:, cols], U[:, cols], THRESHOLD, ALPHA,
                            AOT.is_lt, AOT.mult)
                        vector.drain()
                        vector.tensor_mul(U[:, cols], U[:, cols], KEEP[:, cols])
                        vector.drain()
                        vector.tensor_scalar(
                            THM[:, cols], U[:, cols], -1.0, THRESHOLD,
                            AOT.mult, AOT.add)
                    vector.drain()
                    vector.tensor_add(
                        ACC[:, :], ACC[:, :],
                        STGU[:, (t % 2) * 128:(t % 2) * 128 + 128])
                    if t < nsteps - 1:
                        # all-arithmetic unpack, split by slot-half so rec
                        # j0-3 can start while slots 4-7 still unpack:
                        # hi = (v >= 2) -> odd spike cols; lo = v - 2*hi ->
                        # even cols (packed col q -> spike cols 2q, 2q+1)
                        gp = (t % 2) * 512
                        g8 = (t % 2) * 1024
                        for half, gx, su in (
                                (0, sem_gxa, sem_unp), (1, sem_gxb, sem_unp2)):
                            vector.wait_ge(gx, t + 1)
                            pq = gp + half * 256
                            sq = g8 + half * 512
                            vector.tensor_scalar(
                                SPK8[:, sq + 1:sq + 512:2],
                                SPKP[:, pq:pq + 256], 2.0, None, AOT.is_ge)
                            vector.drain()
                            vector.scalar_tensor_tensor(
                                SPK8[:, sq:sq + 512:2],
                                SPK8[:, sq + 1:sq + 512:2], -2.0,
                                SPKP[:, pq:pq + 256],
                                AOT.mult, AOT.add,
                            ).then_inc(su, 1)
                            vector.drain()
                vector.drain()
                vector.tensor_scalar_mul(
                    OUTS[:, :], ACC[:, :], 1.0 / nsteps
                ).then_inc(sem_init, 1)

    return nc


# ---------------- host side ----------------

def _split(w, nterms):
    terms = []
    rem = w.astype(np.float32)
    for _ in range(nterms):
        t = rem.astype(NPW)
        terms.append(t)
        rem = rem - t.astype(np.float32)
    return terms


def make_in_maps(inputs, W_in, W_rec, nsteps=T, nt=NT):
    inputs = np.asarray(inputs, np.float32)
    W_in = np.asarray(W_in, np.float32)
    W_rec = np.asarray(W_rec, np.float32)
    xtr = np.ascontiguousarray(
        inputs.transpose(1, 2, 0)[:nsteps]
        .reshape(nsteps, 4, 128, B).transpose(0, 2, 1, 3)
        .reshape(nsteps, 128, 4 * B)
    ).astype(NPW)
    in_maps = []
    for c in range(NCORES):
        cols = slice(c * RC, (c + 1) * RC)
        wr_terms = _split(W_rec[:, cols], nt)
        wi_terms = _split(W_in[:, cols], nt)
        m = {"xt": xtr}
        for i, w in enumerate(wr_terms):
            m[f"wr{i}"] = np.ascontiguousarray(w.reshape(16, 128, RC))
        for i, w in enumerate(wi_terms):
            m[f"wi{i}"] = np.ascontiguousarray(w.reshape(4, 128, RC))
        in_maps.append(m)
    return in_maps


def assemble_output(results, nsteps=T):
    rate = np.zeros((B, R), np.float32)
    for c, res in enumerate(results):
        o = np.asarray(res["out"])          # [p, mm*64+b]
        o = o.reshape(128, 2, B).transpose(1, 0, 2).reshape(RC, B)  # [n, b]
        rate[:, c * RC:(c + 1) * RC] = o.T
    return rate


def kernel(inputs, W_in, W_rec):
    from concourse import bass_utils
    nc = build_lsm_nc()
    in_maps = make_in_maps(inputs, W_in, W_rec)
    res = bass_utils.run_bass_kernel_spmd(nc, in_maps, core_ids=list(range(NCORES)))
    return assemble_output(res.results)


# revision 7
# speedup vs baseline: 1.0775x; 1.0775x over previous
"""Liquid State Machine kernel for Trainium2, 8 NeuronCores — v4.

v2 (fp8-direct spike transport) + the latency tricks validated in sim:
- Semaphore bounces: cross-engine waits on DMA/collective completion sems
  observe ~1-1.7us late; a same-engine wait_ge on the completion sem chained
  with .then_inc(fast_sem) releases at completion and propagates in ~100ns.
  Applied to stage-out -> collective, collective -> gather DMAs, and
  gather DMAs -> PE.
- Threshold flip: spikes = (I >= 1 - alpha*u) with THM = 1 - alpha*u''
  precomputed off-critical, so the only DVE op between the last matmul and
  the stage-out DMA is one tensor_tensor is_ge (fp8 out). The membrane
  update (add/reset/decay) runs after the spikes are already in flight.
- Gather DMA split: slots 0-3 on the Activation HWDGE queue, slots 4-7 on
  the SP queue, each ~64KB, with per-half PE gating.
- Spikes cross cores as fp8_e4m3 {0,1} and feed the matmul moving operand
  directly against fp16 stationary weights (mixed-dtype matmul); weights are
  2-term fp16 splits (measured rate rel err 2.5e-3 on this instance).
"""

import numpy as np

from contextlib import ExitStack

import concourse.bass as bass
import concourse.mybir as mybir

ALPHA = 0.9
THRESHOLD = 1.0
B, T, D, R = 64, 256, 512, 2048
NCORES = 8
RC = R // NCORES          # 256 neurons per core
NT = 2                    # fp16 split terms for W_rec / W_in
WDT = mybir.dt.float16
NPW = np.float16
F8 = mybir.dt.float8e4
F32 = mybir.dt.float32
AOT = mybir.AluOpType
XBUF = 4


def build_lsm_nc(nsteps=T, nt=NT):
    nc = bass.Bass(num_devices=NCORES)

    wr = [nc.dram_tensor(f"wr{i}", [16, 128, RC], WDT, kind="ExternalInput")
          for i in range(nt)]
    wi = [nc.dram_tensor(f"wi{i}", [4, 128, RC], WDT, kind="ExternalInput")
          for i in range(nt)]
    xt = nc.dram_tensor("xt", [nsteps, 128, 256], WDT, kind="ExternalInput")
    out = nc.dram_tensor("out", [128, 128], F32, kind="ExternalOutput")
    bin_ = nc.dram_tensor("bounce_in", [128, 64], F8)
    bout = nc.dram_tensor("bounce_out", [NCORES * 128, 64], F8)

    with ExitStack() as ctx:
        WR = ctx.enter_context(nc.sbuf_tensor("WR", [128, nt * 16 * RC], WDT))
        WI = ctx.enter_context(nc.sbuf_tensor("WI", [128, nt * 4 * RC], WDT))
        SPK8 = ctx.enter_context(nc.sbuf_tensor("SPK8", [128, 2 * 1024], F8))
        XT = ctx.enter_context(nc.sbuf_tensor("XT", [128, XBUF * 256], WDT))
        U = ctx.enter_context(nc.sbuf_tensor("U", [128, 128], F32))
        KEEP = ctx.enter_context(nc.sbuf_tensor("KEEP", [128, 128], F32))
        ACC = ctx.enter_context(nc.sbuf_tensor("ACC", [128, 128], F32))
        OUTS = ctx.enter_context(nc.sbuf_tensor("OUTS", [128, 128], F32))
        THM = ctx.enter_context(nc.sbuf_tensor("THM", [128, 128], F32))
        STGU = ctx.enter_context(nc.sbuf_tensor("STGU", [128, 2 * 128], F8))
        PACK8 = ctx.enter_context(nc.sbuf_tensor("PACK8", [128, 2 * 64], F8))
        SPKP = ctx.enter_context(nc.sbuf_tensor("SPKP", [128, 2 * 512], F8))
        PS00 = ctx.enter_context(nc.psum_tensor("PS00", [128, 64], F32))
        PS01 = ctx.enter_context(nc.psum_tensor("PS01", [128, 64], F32))
        PS10 = ctx.enter_context(nc.psum_tensor("PS10", [128, 64], F32))
        PS11 = ctx.enter_context(nc.psum_tensor("PS11", [128, 64], F32))
        sems = {}
        for s in ("sem_w sem_wa sem_fin sem_mm0 sem_mm1 sem_dve sem_u8 sem_stgd "
                  "cc_sem sem_init sem_x0 sem_x1 sem_x2 sem_x3 "
                  "sem_stg2 cc2 sem_gina sem_ginb sem_gxa sem_gxb sem_stgb sem_unp sem_unp2 sem_w2"
                  ).split():
            sems[s] = ctx.enter_context(nc.semaphore(s))
        sem_w, sem_wa, sem_fin = sems["sem_w"], sems["sem_wa"], sems["sem_fin"]
        sem_mm0, sem_mm1 = sems["sem_mm0"], sems["sem_mm1"]
        sem_dve, sem_u8, sem_stgd = sems["sem_dve"], sems["sem_u8"], sems["sem_stgd"]
        cc_sem, sem_init = sems["cc_sem"], sems["sem_init"]
        sem_xb = [sems[f"sem_x{i}"] for i in range(4)]
        sem_stg2, cc2 = sems["sem_stg2"], sems["cc2"]
        sem_gina, sem_ginb = sems["sem_gina"], sems["sem_ginb"]
        sem_gxa, sem_gxb = sems["sem_gxa"], sems["sem_gxb"]
        sem_stgb = sems["sem_stgb"]
        sem_unp = sems["sem_unp"]
        sem_unp2 = sems["sem_unp2"]
        sem_w2 = sems["sem_w2"]
        PS = [[PS00, PS01], [PS10, PS11]]

        def wr_tile(term, q, mm):          # lhsT [128, 128] for W_rec block q
            base = (term * 16 + q) * RC + mm * 128
            return WR[:, base:base + 128]

        def wi_tile(term, dd, mm):
            base = (term * 4 + dd) * RC + mm * 128
            return WI[:, base:base + 128]

        def spk_rhs(buf, j, kk):           # [128, 64] fp8 moving operand
            base = buf * 1024 + j * 128 + kk * 64
            return SPK8[:, base:base + 64]

        def xt_rhs(tb, dd):
            return XT[:, tb * 256 + dd * 64: tb * 256 + dd * 64 + 64]

        with nc.Block() as block:

            @block.sync
            def _(sync):
                for i in range(nt):
                    for dd in range(4):
                        sync.dma_start(
                            WI[:, (i * 4 + dd) * RC:(i * 4 + dd + 1) * RC],
                            wi[i][dd, :, :],
                        ).then_inc(sem_wa, 16)
                for q in range(16):
                    sync.dma_start(
                        WR[:, q * RC:(q + 1) * RC], wr[0][q, :, :],
                    ).then_inc(sem_w, 16)
                for t in range(min(XBUF, nsteps)):
                    sync.dma_start(
                        XT[:, (t % XBUF) * 256:(t % XBUF) * 256 + 256],
                        xt[t, :, :],
                    ).then_inc(sem_xb[t % XBUF], 16)
                for t in range(nsteps):
                    if t < nsteps - 1:
                        # stage-out half 0 (half 1 rides the Act queue)
                        sync.wait_ge(sem_u8, 2 * t + 1)
                        sync.dma_start(
                            bin_[:, 0:32],
                            PACK8[:, (t % 2) * 64:(t % 2) * 64 + 32],
                        ).then_inc(sem_stgd, 16)
                        sync.wait_ge(sem_stgd, 16 * (t + 1)).then_inc(sem_stg2, 1)
                        # gather-in slots 4-7 (slots 0-3 ride the Act queue)
                        sync.wait_ge(cc2, 2 * (t + 1))
                        sync.dma_start(
                            SPKP[:, (t % 2) * 512 + 256:(t % 2) * 512 + 512],
                            bout.ap().rearrange(
                                "(j p) n -> p j n", p=128)[:, 4:8, :],
                        ).then_inc(sem_ginb, 16)
                        sync.wait_ge(sem_ginb, 16 * (t + 1)).then_inc(sem_gxb, 1)
                    tp = t + XBUF
                    if tp < nsteps:
                        sync.wait_ge(sem_mm1, tp - XBUF + 1)
                        sync.dma_start(
                            XT[:, (tp % XBUF) * 256:(tp % XBUF) * 256 + 256],
                            xt[tp, :, :],
                        ).then_inc(sem_xb[tp % XBUF], 16)
                sync.wait_ge(sem_init, 1)
                sync.dma_start(out[:, :], OUTS[:, :]).then_inc(sem_fin, 16)
                sync.wait_ge(sem_fin, 16)

            @block.gpsimd
            def _(g):
                for t in range(nsteps - 1):
                    g.wait_ge(sem_stg2, 2 * (t + 1))
                    g.collective_compute(
                        "AllGather",
                        mybir.AluOpType.bypass,
                        replica_groups=[list(range(NCORES))],
                        ins=[bin_.ap().opt()],
                        outs=[bout.ap().opt()],
                    ).then_inc(cc_sem, 1)
                    g.wait_ge(cc_sem, t + 1).then_inc(cc2, 2)

            @block.scalar
            def _(act):
                for q in range(16):
                    act.dma_start(
                        WR[:, (16 + q) * RC:(16 + q + 1) * RC], wr[1][q, :, :],
                    ).then_inc(sem_w2, 16)
                for t in range(nsteps - 1):
                    # stage-out half 1
                    act.wait_ge(sem_u8, 2 * t + 2)
                    act.dma_start(
                        bin_[:, 32:64],
                        PACK8[:, (t % 2) * 64 + 32:(t % 2) * 64 + 64],
                    ).then_inc(sem_stgb, 16)
                    act.wait_ge(sem_stgb, 16 * (t + 1)).then_inc(sem_stg2, 1)
                    # gather-in slots 0-3
                    act.wait_ge(cc2, 2 * (t + 1))
                    act.dma_start(
                        SPKP[:, (t % 2) * 512:(t % 2) * 512 + 256],
                        bout.ap().rearrange("(j p) n -> p j n", p=128)[:, 0:4, :],
                    ).then_inc(sem_gina, 16)
                    act.wait_ge(sem_gina, 16 * (t + 1)).then_inc(sem_gxa, 1)

            @block.tensor
            def _(tensor):

                def w_in(tau):
                    lw = [None, None]
                    tensor.wait_ge(sem_xb[tau % XBUF], 16 * (tau // XBUF + 1))
                    for mm in range(2):
                        if tau >= 2:
                            tensor.wait_ge(sem_dve, 2 * (tau - 2) + mm + 1)
                        for dd in range(4):
                            for i in range(nt):
                                lw[mm] = tensor.matmul(
                                    PS[tau % 2][mm][:, :],
                                    wi_tile(i, dd, mm),
                                    xt_rhs(tau % XBUF, dd),
                                    start=(dd == 0 and i == 0),
                                    stop=(tau == 0 and dd == 3 and i == nt - 1),
                                )
                    return lw

                tensor.wait_ge(sem_wa, 16 * nt * 4)
                last = w_in(0)
                for t in range(nsteps):
                    buf = t % 2
                    ps = PS[buf]
                    if t >= 1:
                        if t == 1:
                            tensor.wait_ge(sem_w, 16 * 16)
                            tensor.wait_ge(sem_w2, 16 * 16)
                        for mm in range(2):
                            for j in range(NCORES):
                                if mm == 0 and j == 0:
                                    tensor.wait_ge(sem_unp, t)
                                if mm == 0 and j == 4:
                                    tensor.wait_ge(sem_unp2, t)
                                for kk in range(2):
                                    q = 2 * j + kk
                                    for i in range(nt):
                                        last[mm] = tensor.matmul(
                                            ps[mm][:, :],
                                            wr_tile(i, q, mm),
                                            spk_rhs((t - 1) % 2, j, kk),
                                            start=False,
                                            stop=(kk == 1 and j == NCORES - 1
                                                  and i == nt - 1),
                                        )
                    last[0].then_inc(sem_mm0, 1)
                    last[1].then_inc(sem_mm1, 1)
                    if t + 1 < nsteps:
                        tensor.wait_ge(cc2, 2 * (t + 1))
                        last = w_in(t + 1)

            @block.vector
            def _(vector):
                vector.memset(U[:, :], 0.0)
                vector.memset(ACC[:, :], 0.0)
                vector.memset(THM[:, :], THRESHOLD)
                vector.drain()
                for t in range(nsteps):
                    ps = PS[t % 2]
                    # critical: one is_ge per half (fp8 out), straight to DMA
                    if t >= 2:
                        vector.wait_ge(sem_stg2, 2 * (t - 1))
                    for h in range(2):
                        cols = slice(h * 64, h * 64 + 64)
                        sb = (t % 2) * 128 + h * 64
                        vector.wait_ge([sem_mm0, sem_mm1][h], t + 1)
                        vector.tensor_tensor(
                            STGU[:, sb:sb + 64],
                            ps[h][:, :], THM[:, cols], AOT.is_ge,
                        )
                        vector.drain()
                        # pack 2 spikes/byte: v = s_even + 2*s_odd in {0..3}
                        pb = (t % 2) * 64 + h * 32
                        vector.scalar_tensor_tensor(
                            PACK8[:, pb:pb + 32],
                            STGU[:, sb + 1:sb + 64:2], 2.0,
                            STGU[:, sb:sb + 64:2],
                            AOT.mult, AOT.add,
                        ).then_inc(sem_u8, 1)
                    # off-critical: membrane update + next threshold + count
                    for h in range(2):
                        cols = slice(h * 64, h * 64 + 64)
                        vector.tensor_add(
                            U[:, cols], U[:, cols], ps[h][:, :]
                        ).then_inc(sem_dve, 1)
                        vector.drain()
                        vector.tensor_scalar(
                            KEEP[:, cols], U[:, cols], THRESHOLD, ALPHA,
                            AOT.is_lt, AOT.mult)
                        vector.drain()
                        vector.tensor_mul(U[:, cols], U[:, cols], KEEP[:, cols])
                        vector.drain()
                        vector.tensor_scalar(
                            THM[:, cols], U[:, cols], -1.0, THRESHOLD,
                            AOT.mult, AOT.add)
                    vector.drain()
                    vector.tensor_add(
                        ACC[:, :], ACC[:, :],
                        STGU[:, (t % 2) * 128:(t % 2) * 128 + 128])
                    if t < nsteps - 1:
                        # all-arithmetic unpack, split by slot-half so rec
                        # j0-3 can start while slots 4-7 still unpack:
                        # hi = (v >= 2) -> odd spike cols; lo = v - 2*hi ->
                        # even cols (packed col q -> spike cols 2q, 2q+1)
                        gp = (t % 2) * 512
                        g8 = (t % 2) * 1024
                        for half, gx, su in (
                                (0, sem_gxa, sem_unp), (1, sem_gxb, sem_unp2)):
                            vector.wait_ge(gx, t + 1)
                            pq = gp + half * 256
                            sq = g8 + half * 512
                            vector.tensor_scalar(
                                SPK8[:, sq + 1:sq + 512:2],
                                SPKP[:, pq:pq + 256], 2.0, None, AOT.is_ge)
                            vector.drain()
                            vector.scalar_tensor_tensor(
                                SPK8[:, sq:sq + 512:2],
                                SPK8[:, sq + 1:sq + 512:2], -2.0,
                                SPKP[:, pq:pq + 256],
                                AOT.mult, AOT.add,
                            ).then_inc(su, 1)
                            vector.drain()
                vector.drain()
                vector.tensor_scalar_mul(
                    OUTS[:, :], ACC[:, :], 1.0 / nsteps
                ).then_inc(sem_init, 1)

    return nc


# ---------------- host side ----------------

def _split(w, nterms):
    terms = []
    rem = w.astype(np.float32)
    for _ in range(nterms):
        t = rem.astype(NPW)
        terms.append(t)
        rem = rem - t.astype(np.float32)
    return terms


def make_in_maps(inputs, W_in, W_rec, nsteps=T, nt=NT):
    inputs = np.asarray(inputs, np.float32)
    W_in = np.asarray(W_in, np.float32)
    W_rec = np.asarray(W_rec, np.float32)
    xtr = np.ascontiguousarray(
        inputs.transpose(1, 2, 0)[:nsteps]
        .reshape(nsteps, 4, 128, B).transpose(0, 2, 1, 3)
        .reshape(nsteps, 128, 4 * B)
    ).astype(NPW)
    in_maps = []
    for c in range(NCORES):
        cols = slice(c * RC, (c + 1) * RC)
        wr_terms = _split(W_rec[:, cols], nt)
        wi_terms = _split(W_in[:, cols], nt)
        m = {"xt": xtr}
        for i, w in enumerate(wr_terms):
            m[f"wr{i}"] = np.ascontiguousarray(w.reshape(16, 128, RC))
        for i, w in enumerate(wi_terms):
            m[f"wi{i}"] = np.ascontiguousarray(w.reshape(4, 128, RC))
        in_maps.append(m)
    return in_maps


def assemble_output(results, nsteps=T):
    rate = np.zeros((B, R), np.float32)
    for c, res in enumerate(results):
        o = np.asarray(res["out"])          # [p, mm*64+b]
        o = o.reshape(128, 2, B).transpose(1, 0, 2).reshape(RC, B)  # [n, b]
        rate[:, c * RC:(c + 1) * RC] = o.T
    return rate


def kernel(inputs, W_in, W_rec):
    from concourse import bass_utils
    nc = build_lsm_nc()
    in_maps = make_in_maps(inputs, W_in, W_rec)
    res = bass_utils.run_bass_kernel_spmd(nc, in_maps, core_ids=list(range(NCORES)))
    return assemble_output(res.results)
